# revision 47
# baseline (speedup 1.0000x reference)
"""Trainium2 Bass kernel for CandidateFinder (retrieval_knn).

Math: for each (batch, query row), candidates = the K_MAX=64 smallest key
indices whose 32-dim sign pattern matches the query's in either dim-group
(dims 0:32 or 32:64), ascending, padded with -1.

Structure: a fast SCREEN kernel computes exact per-span any-match
statistics (zero false negatives: exact sign quantize to +-0.5 bf16 on
host, exact fp32 dots on PE, match <=> dot == 8, best non-match 7.5)
plus the all-(-1) output.  The host inspects the device-computed
statistics and only if a match exists launches the EXACT kernel (lazily
compiled) to rewrite the output.  With random inputs a 32-bit sign
collision has probability ~2^-32 per pair, so the screen path is the
only one that runs; the exact path keeps kernel() correct for any input.

Screen kernel (per core = batch b, query half): raw Bass, hand-placed
semaphores.  PSUM ([128,4096] f32 = 8 banks) is split into two fully
DECOUPLED depth-2 rings so neither detection engine ever waits on the
other: DVE owns banks 0-3 (16 spans of 1024 dot-columns, all h=0 work,
PE row-groups 0/32), ACT owns banks 4-7 (16 spans, h=1 work, row-groups
64/96) -- a D-fill and an A-fill always use disjoint row-groups and run
concurrently on the PE (4x 32-row quadrant matmuls, K=32, N=512).  Both
engines run back-to-back with zero stalls: DVE detects via
tensor_scalar is_ge(7.9) with sum-accumulator (match count per span,
~1145ns/span incl. hidden 82ns accumulator readout); ACT detects via
Relu activation, bias=-632, scale=80 (sum of relu(80*(d-7.9)),
~1185ns/span); ACT is ~40ns/span slower so DVE also takes the second
half of the last ACT span, equalising the finish times.  Per-queue DMA
chunks cost ~2.4us each regardless of size (latency-dominated), so
inputs ship as FOUR big chunks over the 3 DMA-capable queues, with
query-slab-0 spliced into the first ks chunk
([k 0:512 | q-slab0 | k 512:1024]) so the first spans' data arrives in
one early chunk.  Inputs are +-0.5 bf16 (host sign quantize, exact;
dot == 8 iff 32-dim sign match, best non-match 7.5).  The final flag
DMA's completion is not waited on: the NEFF's fixed per-engine
semaphore-reset postamble (~7.5us, the dominant fixed cost) covers the
receipt.  exec breakdown: ~4.2us DMA lead-in + ~19.0us detection +
~0.7us flag + ~7.5us postamble ~= 31.5us (baseline: 38.1us).
"""

import numpy as np

import concourse.bacc as bacc
import concourse.mybir as mybir
from concourse.tile import TileContext
from concourse import bass_utils

B, L, D = 4, 2048, 64
HALF = 1024          # query rows per core
N_CORES = 8
K_MAX = 64
QT = HALF // 128     # 8 query slabs per core
THRESH = 7.9         # between 7.5 (best non-match) and 8.0 (match)
SENT = 4096.0        # sentinel > any index (exact kernel)

f32 = mybir.dt.float32
bf16 = mybir.dt.bfloat16
i32 = mybir.dt.int32
u32 = mybir.dt.uint32
Alu = mybir.AluOpType
Ax = mybir.AxisListType
AF = mybir.ActivationFunctionType

_CACHE = {}

N_DSPAN = 16         # DVE 1024-col spans (banks 0-3 ring)
N_ASPAN = 16         # ACT 1024-col spans (banks 4-7 ring)
N_STAT = 33          # flag columns: [0:16]=DVE counts, [16:32]=ACT sums


def _build_screen():
    nc = bacc.Bacc("TRN2", target_bir_lowering=False,
                   enable_partition_id=False)
    # qs[d + 64*dup, i] = sign(q[half*1024 + i, d]) * 0.5   (dims-major, dup)
    qs = nc.dram_tensor("qs", [128, HALF], bf16, kind="ExternalInput")
    # ks[d + 64*h, j] = sign(k[h*1024 + j, d]) * 0.5 (halves stacked), with
    # qs slab 0 spliced in at cols 512:640 so the first span's data ships
    # as one early DMA chunk: [k 0:512 | q-slab0 | k 512:1024]
    ks = nc.dram_tensor("ks", [128, 1152], bf16, kind="ExternalInput")
    out = nc.dram_tensor("out", [HALF, K_MAX], i32, kind="ExternalOutput")
    flag = nc.dram_tensor("flag", [128, N_STAT], f32, kind="ExternalOutput")
    out_pt = out[:].rearrange("(p t) c -> p (t c)", p=128)

    # --- memory ---
    qsb = nc.alloc_sbuf_tensor("qsb", [128, HALF], bf16)
    ksb = nc.alloc_sbuf_tensor("ksb", [128, 1152], bf16)
    run = nc.alloc_sbuf_tensor("run", [128, 1024], f32)
    scr = nc.alloc_sbuf_tensor("scr", [128, 2048], f32)
    stat = nc.alloc_sbuf_tensor("stat", [128, N_STAT], f32)
    rbias = nc.alloc_sbuf_tensor("rbias", [128, 1], f32)
    out_sb = nc.alloc_sbuf_tensor("out_sb", [128, QT * K_MAX], i32)
    ps = nc.alloc_psum_tensor("ps", [128, 4096], f32)

    names = ["s_k", "s_k2", "s_q1", "s_q3", "s_ms",
             "s_od", "s_fd", "s_fa", "s_dd", "s_da"]
    sem = {n: nc.alloc_semaphore(n) for n in names}
    (s_k, s_k2, s_q1, s_q3, s_ms, s_od, s_fd, s_fa,
     s_dd, s_da) = (sem[n] for n in names)

    # --- input DMA: per-chunk cost on a queue is latency-dominated
    #     (~2.4us each regardless of size), so ship as FEW chunks as the
    #     arrival deadlines allow; the first chunk carries kb0 + q-slab0 ---
    nc.sync.dma_start(ksb[:, 0:640], ks[:, 0:640]).then_inc(s_k, 16)
    nc.scalar.dma_start(ksb[:, 640:1152],
                        ks[:, 640:1152]).then_inc(s_k2, 16)
    nc.scalar.dma_start(qsb[:, 128:512], qs[:, 128:512]).then_inc(s_q1, 16)
    nc.sync.dma_start(qsb[:, 512:1024],
                      qs[:, 512:1024]).then_inc(s_q3, 16)
    nc.gpsimd.memset(out_sb[:, :], -1).then_inc(s_ms)

    # --- small constants (cheap, off the critical path) ---
    nc.vector.memset(stat[:, :], 0.0)
    nc.vector.memset(rbias[:, :], -632.0).then_inc(s_ms)  # 80*7.9 exact

    # --- out: all -1, DMA'd early; receipt gates the final flag ---
    nc.sync.wait_ge(s_ms, 2)
    nc.sync.dma_start(out_pt, out_sb[:, :]).then_inc(s_od, 16)

    # --- PE: 64 matmuls (K=32, N=512, 4 concurrent row-groups by (h,g));
    #     each span fill = 2 matmuls (dim-groups g0,g1), inc on the 2nd ---
    def mm(bank, slab, h, g, kb, waits=(), inc=None):
        r0 = 64 * h + 32 * g
        for s, v in waits:
            nc.tensor.wait_ge(s, v)
        kcol = 640 * kb   # kb0 at ksb cols 0:512, kb1 at 640:1152
        ins = nc.tensor.matmul(
            ps[:, 512 * bank:512 * bank + 512],
            lhsT=qs_sb_slab(slab, r0),
            rhs=ksb[r0:r0 + 32, kcol:kcol + 512],
            start=True, stop=True, tile_position=(r0, 0))
        if inc is not None:
            ins.then_inc(inc)

    def qs_sb_slab(slab, r0):
        if slab == 0:                   # spliced into the first ks chunk
            return ksb[r0:r0 + 32, 512:640]
        return qsb[r0:r0 + 32, 128 * slab:128 * slab + 128]

    # Two decoupled depth-2 rings: DVE owns banks 0-3 (spans D0..D15,
    # h=0 work, PE row-groups 0/32), ACT owns banks 4-7 (spans A0..A15,
    # h=1 work, row-groups 64/96) — so a D-fill and an A-fill always use
    # disjoint row-groups and run concurrently on the PE.
    # Span i covers (slab=i//2, kb=i%2), both dim-groups.
    for i in range(16):
        slab, kb = i // 2, i % 2
        w = []
        if i == 0:
            w = [(s_k, 16)]
        elif i == 1:
            w = [(s_k2, 16)]
        elif i == 2:
            w = [(s_q1, 16)]
        elif i == 8:
            w = [(s_q3, 16)]
        if i >= 2:
            w.append((s_dd, i - 1))
        b = 2 * (i % 2)                 # D-span banks
        mm(b, slab, 0, 0, kb, waits=w)
        mm(b + 1, slab, 0, 1, kb, inc=s_fd)
        wa = [(s_da, i - 1)] if i >= 2 else ()
        mm(4 + b, slab, 1, 0, kb, waits=wa)
        mm(4 + b + 1, slab, 1, 1, kb, inc=s_fa)

    # --- DVE: per-span is_ge(7.9) indicator with sum-accumulator.
    #     ACT runs ~40ns/span slower, so DVE also takes the second half
    #     of the last ACT span (A15) to equalise the finish times. ---
    DCOL = [slice(0, 1024), slice(1024, 2048)]
    ACOL = [slice(2048, 3072), slice(3072, 4096)]
    for j in range(16):                 # D-span index
        nc.vector.wait_ge(s_fd, j + 1)
        nc.vector.tensor_scalar(run[:, :], ps[:, DCOL[j % 2]], 7.9, 0.0,
                                op0=Alu.is_ge, op1=Alu.add,
                                accum_out=stat[:, j:j + 1]).then_inc(s_dd)
    nc.vector.wait_ge(s_fa, 16)
    nc.vector.tensor_scalar(run[:, 0:512], ps[:, 3584:4096], 7.9, 0.0,
                            op0=Alu.is_ge, op1=Alu.add,
                            accum_out=stat[:, 32:33]).then_inc(s_dd)

    # --- ACT: relu-accumulate over its spans (only half of A15) ---
    nc.scalar.wait_ge(s_ms, 2)          # rbias + stat initialised
    for i in range(16):
        nc.scalar.wait_ge(s_fa, i + 1)
        if i == 15:
            nc.scalar.activation(scr[:, 0:512], ps[:, 3072:3584], AF.Relu,
                                 bias=rbias[:, 0:1], scale=80.0,
                                 accum_out=stat[:, 31:32]).then_inc(s_da)
        else:
            nc.scalar.activation(scr[:, 0:1024], ps[:, ACOL[i % 2]],
                                 AF.Relu, bias=rbias[:, 0:1], scale=80.0,
                                 accum_out=stat[:, 16 + i:17 + i]
                                 ).then_inc(s_da)

    # --- flag (triggered by scalar): after out landed + ALL stats have
    #     been read out (incl. its own accumulator readouts, which walrus
    #     may schedule past later sequencer work); no completion wait ---
    nc.scalar.wait_ge(s_da, N_ASPAN)
    nc.scalar.wait_ge(s_od, 16)
    nc.scalar.wait_ge(s_dd, 17)
    nc.scalar.dma_start(flag[:], stat[:, :]).then_inc(s_od, 16)

    nc.compile()
    return nc


def get_nc():
    if "nc" not in _CACHE:
        _CACHE["nc"] = _build_screen()
    return _CACHE["nc"]


def make_in_maps(query_up, key_up):
    """Sign quantize (exact +-0.5 bf16) + pure layout transforms."""
    import ml_dtypes
    query_up = np.asarray(query_up, dtype=np.float32)
    key_up = np.asarray(key_up, dtype=np.float32)
    bf = ml_dtypes.bfloat16
    in_maps = []
    for c in range(N_CORES):
        b, half = c // 2, c % 2
        q = query_up[b, half * HALF:(half + 1) * HALF]       # [1024, 64]
        qT = np.where(q > 0, 0.5, -0.5).astype(bf).T         # [64, 1024]
        qsm = np.ascontiguousarray(np.concatenate([qT, qT], axis=0))
        kT = np.where(key_up[b] > 0, 0.5, -0.5).astype(bf).T  # [64, 2048]
        km = np.concatenate([kT[:, 0:1024], kT[:, 1024:2048]], axis=0)
        # [k 0:512 | q-slab0 | k 512:1024] (see _build_screen)
        ksm = np.ascontiguousarray(np.concatenate(
            [km[:, 0:512], qsm[:, 0:128], km[:, 512:1024]], axis=1))
        in_maps.append({"qs": qsm, "ks": ksm})
    return in_maps


# ---------------------------------------------------------------------------
# Exact kernel (full implementation) -- only compiled and run if the screen
# statistics fire, i.e. some query/key pair shares a 32-bit sign pattern.
# ---------------------------------------------------------------------------


def _build_exact():
    nc = bacc.Bacc("TRN2", target_bir_lowering=False,
                   enable_partition_id=False)
    # qt4[h*64+d, pair*128+p] = q[p*8 + 2*pair + h, d]
    qt4 = nc.dram_tensor("qt4", [128, HALF // 2], f32, kind="ExternalInput")
    # kt4[dup*64+d, j] = k[j, d]
    kt4 = nc.dram_tensor("kt4", [128, L], f32, kind="ExternalInput")
    out = nc.dram_tensor("out", [HALF, K_MAX], i32, kind="ExternalOutput")
    out_pt = out[:].rearrange("(p t) c -> p (t c)", p=128)

    with TileContext(nc) as tc:
        with tc.tile_pool(name="sb", bufs=1) as sb, \
             tc.tile_pool(name="sb2", bufs=3) as sb2, \
             tc.tile_pool(name="ps", bufs=2, space="PSUM") as ps:

            qsb = sb.tile([128, HALF // 2], f32)
            ksb = sb.tile([128, L], f32)
            sqT4 = sb.tile([128, HALF // 2], bf16)
            skT4 = sb.tile([128, L], bf16)
            nc.default_dma_engine.dma_start(ksb[:, 0:1024], kt4[:, 0:1024])
            nc.scalar.dma_start(ksb[:, 1024:2048], kt4[:, 1024:2048])
            nc.default_dma_engine.dma_start(qsb, qt4[:, :])
            nc.vector.tensor_scalar(skT4, ksb, 0.0, 0.5,
                                    op0=Alu.is_gt, op1=Alu.subtract)
            nc.vector.tensor_scalar(sqT4, qsb, 0.0, 0.5,
                                    op0=Alu.is_gt, op1=Alu.subtract)

            out_sb = sb.tile([128, QT * K_MAX], i32)
            nc.gpsimd.memset(out_sb, -1)

            c2i = sb.tile([128, L], i32)   # SENT - j (key j = column)
            nc.gpsimd.iota(c2i, pattern=[[-1, L]], base=int(SENT),
                           channel_multiplier=0)
            c2f = sb.tile([128, L], f32)
            nc.gpsimd.tensor_copy(c2f, c2i)
            negone = sb.tile([128, K_MAX], f32)
            nc.vector.memset(negone, -1.0)
            for t in range(QT):
                base = (t % 2) * 64
                qc = slice((t // 2) * 128, (t // 2) * 128 + 128)
                lhs0 = sqT4[base:base + 32, qc]
                lhs1 = sqT4[base + 32:base + 64, qc]
                val = sb.tile([128, L], f32, tag="val")
                for h in range(2):
                    p0 = ps.tile([128, 1024], f32, tag="g0")
                    p1 = ps.tile([128, 1024], f32, tag="g1")
                    for sblk in range(2):
                        kc = slice(h * 1024 + sblk * 512,
                                   h * 1024 + (sblk + 1) * 512)
                        sl = slice(sblk * 512, (sblk + 1) * 512)
                        nc.tensor.matmul(p0[:, sl], lhsT=lhs0,
                                         rhs=skT4[base:base + 32, kc],
                                         start=True, stop=True,
                                         tile_position=(base, 0))
                        nc.tensor.matmul(p1[:, sl], lhsT=lhs1,
                                         rhs=skT4[base + 32:base + 64, kc],
                                         start=True, stop=True,
                                         tile_position=(base + 32, 0))
                    hsl = slice(h * 1024, (h + 1) * 1024)
                    m0 = sb2.tile([128, 1024], f32, tag="m0")
                    nc.vector.tensor_scalar(m0, p0, THRESH,
                                            None, op0=Alu.is_ge)
                    m1 = sb2.tile([128, 1024], f32, tag="m1")
                    nc.vector.scalar_tensor_tensor(
                        m1, in0=p1, scalar=THRESH, in1=m0,
                        op0=Alu.is_ge, op1=Alu.max)
                    # val = m1 ? -(j) : -SENT  ==  m1*(SENT-j) - SENT
                    nc.vector.tensor_tensor(
                        out=val[:, hsl], in0=m1, in1=c2f[:, hsl],
                        op=Alu.mult)
                    nc.vector.tensor_scalar_add(val[:, hsl], val[:, hsl],
                                                -SENT)
                # 64 smallest j == 64 largest of val, descending
                no = sb.tile([128, K_MAX], f32, tag="no")
                for it8 in range(8):
                    osl = slice(it8 * 8, (it8 + 1) * 8)
                    nc.vector.max(out=no[:, osl], in_=val)
                    nc.vector.match_replace(
                        out=val, in_to_replace=no[:, osl],
                        in_values=val, imm_value=-SENT)
                jv = sb.tile([128, K_MAX], f32, tag="jv")
                nc.vector.tensor_scalar_mul(jv, no, -1.0)  # j or SENT
                msk = sb.tile([128, K_MAX], u32, tag="msk")
                nc.vector.tensor_scalar(msk, jv, 2048.5, None,
                                        op0=Alu.is_ge)
                nc.vector.copy_predicated(jv, msk, negone)
                nc.vector.tensor_copy(
                    out_sb[:, t * K_MAX:(t + 1) * K_MAX], jv)

            nc.default_dma_engine.dma_start(out_pt, out_sb)

    nc.compile()
    return nc


def get_nc_exact():
    if "nc_exact" not in _CACHE:
        _CACHE["nc_exact"] = _build_exact()
    return _CACHE["nc_exact"]


def make_in_maps_exact(query_up, key_up):
    query_up = np.asarray(query_up, dtype=np.float32)
    key_up = np.asarray(key_up, dtype=np.float32)
    in_maps = []
    for c in range(N_CORES):
        b, half = c // 2, c % 2
        q = query_up[b, half * HALF:(half + 1) * HALF]       # [1024, 64]
        qt4 = np.ascontiguousarray(
            q.reshape(128, 4, 2, D).transpose(2, 3, 1, 0).reshape(
                128, HALF // 2))
        kT = key_up[b].T                                     # [64, 2048]
        kt4 = np.ascontiguousarray(np.concatenate([kT, kT], axis=0))
        in_maps.append({"qt4": qt4, "kt4": kt4})
    return in_maps


def _flag_fires(flag):
    return ((flag[:, 0:16] >= 0.5).any() or (flag[:, 32:33] >= 0.5).any()
            or (flag[:, 16:32] >= 1.0).any())


def _run_exact(query_up, key_up):
    """Run the exact kernel in a fresh subprocess: this runtime cannot
    load a second, different NEFF in one process."""
    import os
    import subprocess
    import sys
    import tempfile
    d = tempfile.mkdtemp(prefix="cand_exact_")
    np.save(os.path.join(d, "q.npy"), np.asarray(query_up, dtype=np.float32))
    np.save(os.path.join(d, "k.npy"), np.asarray(key_up, dtype=np.float32))
    here = os.path.dirname(os.path.abspath(__file__))
    code = (
        "import sys, numpy as np\n"
        f"sys.path.insert(0, {here!r})\n"
        "import kernel as K\n"
        "from concourse import bass_utils\n"
        f"q = np.load({os.path.join(d, 'q.npy')!r})\n"
        f"k = np.load({os.path.join(d, 'k.npy')!r})\n"
        "res = bass_utils.run_bass_kernel_spmd(\n"
        "    K.get_nc_exact(), K.make_in_maps_exact(q, k),\n"
        "    core_ids=list(range(K.N_CORES)))\n"
        "np.save(" + repr(os.path.join(d, "out.npy")) + ",\n"
        "        np.stack([res.results[c]['out']\n"
        "                  for c in range(K.N_CORES)]))\n"
    )
    subprocess.run([sys.executable, "-c", code], check=True)
    return np.load(os.path.join(d, "out.npy"))


def kernel(query_up, key_up, head_idx=None, **_ignored):
    nc = get_nc()
    in_maps = make_in_maps(query_up, key_up)
    res = bass_utils.run_bass_kernel_spmd(
        nc, in_maps, core_ids=list(range(N_CORES)))
    full = np.empty((B, L, K_MAX), dtype=np.int32)
    if any(_flag_fires(res.results[c]["flag"]) for c in range(N_CORES)):
        # rare: some pair shares a full 32-bit sign pattern -> exact kernel
        outs = _run_exact(query_up, key_up)
        for c in range(N_CORES):
            b, half = c // 2, c % 2
            full[b, half * HALF:(half + 1) * HALF] = outs[c]
    else:
        for c in range(N_CORES):
            b, half = c // 2, c % 2
            full[b, half * HALF:(half + 1) * HALF] = res.results[c]["out"]
    return full


# revision 48
# speedup vs baseline: 1.0479x; 1.0479x over previous
"""Trainium2 Bass kernel for CandidateFinder (retrieval_knn).

Math: for each (batch, query row), candidates = the K_MAX=64 smallest key
indices whose 32-dim sign pattern matches the query's in either dim-group
(dims 0:32 or 32:64), ascending, padded with -1.

Structure: a fast SCREEN kernel computes exact per-span any-match
statistics (zero false negatives: exact sign quantize to +-0.5 bf16 on
host, exact fp32 dots on PE, match <=> dot == 8, best non-match 7.5)
plus the all-(-1) output.  The host inspects the device-computed
statistics and only if a match exists launches the EXACT kernel (lazily
compiled) to rewrite the output.  With random inputs a 32-bit sign
collision has probability ~2^-32 per pair, so the screen path is the
only one that runs; the exact path keeps kernel() correct for any input.

Screen kernel (per core = batch b, query half): raw Bass, hand-placed
semaphores.  PSUM ([128,4096] f32 = 8 banks) is split into two fully
DECOUPLED depth-2 rings so neither detection engine ever waits on the
other: DVE owns banks 0-3 (16 spans of 1024 dot-columns, all h=0 work,
PE row-groups 0/32), ACT owns banks 4-7 (16 spans, h=1 work, row-groups
64/96) -- a D-fill and an A-fill always use disjoint row-groups and run
concurrently on the PE (4x 32-row quadrant matmuls, K=32, N=512).  Both
engines run back-to-back with zero stalls: DVE detects via
tensor_scalar is_ge(7.9) with sum-accumulator (match count per span,
~1145ns/span incl. hidden 82ns accumulator readout); ACT detects via
Relu activation, bias=-632, scale=80 (sum of relu(80*(d-7.9)),
~1185ns/span); ACT is ~40ns/span slower so DVE also takes the second
half of the last ACT span, equalising the finish times.  Per-queue DMA
chunks cost ~2.4us each regardless of size (latency-dominated), so
inputs ship as FOUR big chunks over the 3 DMA-capable queues, with
query-slab-0 spliced into the first ks chunk
([k 0:512 | q-slab0 | k 512:1024]) so the first spans' data arrives in
one early chunk.  Inputs are +-0.5 bf16 (host sign quantize, exact;
dot == 8 iff 32-dim sign match, best non-match 7.5).  The final flag
DMA's completion is not waited on: the NEFF's fixed per-engine
semaphore-reset postamble (~7.5us, the dominant fixed cost) covers the
receipt.  exec breakdown: ~4.2us DMA lead-in + ~19.0us detection +
~0.7us flag + ~7.5us postamble ~= 31.5us (baseline: 38.1us).
"""

import numpy as np

import concourse.bacc as bacc
import concourse.mybir as mybir
from concourse.tile import TileContext
from concourse import bass_utils

B, L, D = 4, 2048, 64
HALF = 1024          # query rows per core
N_CORES = 8
K_MAX = 64
QT = HALF // 128     # 8 query slabs per core
THRESH = 7.9         # between 7.5 (best non-match) and 8.0 (match)
SENT = 4096.0        # sentinel > any index (exact kernel)

f32 = mybir.dt.float32
bf16 = mybir.dt.bfloat16
f8 = mybir.dt.float8e4
i32 = mybir.dt.int32
u32 = mybir.dt.uint32
Alu = mybir.AluOpType
Ax = mybir.AxisListType
AF = mybir.ActivationFunctionType

_CACHE = {}

N_DSPAN = 16         # DVE 1024-col spans (banks 0-3 ring)
N_ASPAN = 16         # ACT 1024-col spans (banks 4-7 ring)
N_STAT = 33          # flag columns: [0:16]=DVE counts, [16:32]=ACT sums


def _build_screen():
    nc = bacc.Bacc("TRN2", target_bir_lowering=False,
                   enable_partition_id=False)
    # qs[d + 64*dup, i] = sign(q[half*1024 + i, d]) * 0.5   (dims-major, dup)
    qs = nc.dram_tensor("qs", [128, HALF], f8, kind="ExternalInput")
    # ks[d + 64*h, j] = sign(k[h*1024 + j, d]) * 0.5 (halves stacked), with
    # qs slab 0 spliced in at cols 512:640 so the first span's data ships
    # as one early DMA chunk: [k 0:512 | q-slab0 | k 512:1024]
    ks = nc.dram_tensor("ks", [128, 1152], f8, kind="ExternalInput")
    out = nc.dram_tensor("out", [HALF, K_MAX], i32, kind="ExternalOutput")
    flag = nc.dram_tensor("flag", [128, N_STAT], f32, kind="ExternalOutput")
    out_pt = out[:].rearrange("(p t) c -> p (t c)", p=128)

    # --- memory ---
    qsb = nc.alloc_sbuf_tensor("qsb", [128, HALF], f8)
    ksb = nc.alloc_sbuf_tensor("ksb", [128, 1152], f8)
    run = nc.alloc_sbuf_tensor("run", [128, 1024], f32)
    scr = nc.alloc_sbuf_tensor("scr", [128, 2048], f32)
    stat = nc.alloc_sbuf_tensor("stat", [128, N_STAT], f32)
    rbias = nc.alloc_sbuf_tensor("rbias", [128, 1], f32)
    out_sb = nc.alloc_sbuf_tensor("out_sb", [128, QT * K_MAX], i32)
    ps = nc.alloc_psum_tensor("ps", [128, 4096], f32)

    names = ["s_k", "s_k2", "s_q1", "s_q3", "s_ms",
             "s_od", "s_fd", "s_fa", "s_dd", "s_da"]
    sem = {n: nc.alloc_semaphore(n) for n in names}
    (s_k, s_k2, s_q1, s_q3, s_ms, s_od, s_fd, s_fa,
     s_dd, s_da) = (sem[n] for n in names)

    # --- input DMA: per-chunk cost on a queue is latency-dominated
    #     (~2.4us each regardless of size), so ship as FEW chunks as the
    #     arrival deadlines allow; the first chunk carries kb0 + q-slab0 ---
    nc.sync.dma_start(ksb[:, 0:640], ks[:, 0:640]).then_inc(s_k, 16)
    nc.scalar.dma_start(ksb[:, 640:1152],
                        ks[:, 640:1152]).then_inc(s_k2, 16)
    nc.scalar.dma_start(qsb[:, 128:512], qs[:, 128:512]).then_inc(s_q1, 16)
    nc.sync.dma_start(qsb[:, 512:1024],
                      qs[:, 512:1024]).then_inc(s_q3, 16)
    nc.gpsimd.memset(out_sb[:, :], -1).then_inc(s_ms)

    # --- small constants (cheap, off the critical path) ---
    nc.vector.memset(stat[:, :], 0.0)
    nc.vector.memset(rbias[:, :], -632.0).then_inc(s_ms)  # 80*7.9 exact

    # --- out: all -1, DMA'd early; receipt gates the final flag ---
    nc.sync.wait_ge(s_ms, 2)
    nc.sync.dma_start(out_pt, out_sb[:, :]).then_inc(s_od, 16)

    # --- PE: 64 matmuls (K=32, N=512, 4 concurrent row-groups by (h,g));
    #     each span fill = 2 matmuls (dim-groups g0,g1), inc on the 2nd ---
    def mm(bank, slab, h, g, kb, waits=(), inc=None):
        r0 = 64 * h + 32 * g
        for s, v in waits:
            nc.tensor.wait_ge(s, v)
        kcol = 640 * kb   # kb0 at ksb cols 0:512, kb1 at 640:1152
        ins = nc.tensor.matmul(
            ps[:, 512 * bank:512 * bank + 512],
            lhsT=qs_sb_slab(slab, r0),
            rhs=ksb[r0:r0 + 32, kcol:kcol + 512],
            start=True, stop=True, tile_position=(r0, 0))
        if inc is not None:
            ins.then_inc(inc)

    def qs_sb_slab(slab, r0):
        if slab == 0:                   # spliced into the first ks chunk
            return ksb[r0:r0 + 32, 512:640]
        return qsb[r0:r0 + 32, 128 * slab:128 * slab + 128]

    # Two decoupled depth-2 rings: DVE owns banks 0-3 (spans D0..D15,
    # h=0 work, PE row-groups 0/32), ACT owns banks 4-7 (spans A0..A15,
    # h=1 work, row-groups 64/96) — so a D-fill and an A-fill always use
    # disjoint row-groups and run concurrently on the PE.
    # Span i covers (slab=i//2, kb=i%2), both dim-groups.
    for i in range(16):
        slab, kb = i // 2, i % 2
        w = []
        if i == 0:
            w = [(s_k, 16)]
        elif i == 1:
            w = [(s_k2, 16)]
        elif i == 2:
            w = [(s_q1, 16)]
        elif i == 8:
            w = [(s_q3, 16)]
        if i >= 2:
            w.append((s_dd, i - 1))
        b = 2 * (i % 2)                 # D-span banks
        mm(b, slab, 0, 0, kb, waits=w)
        mm(b + 1, slab, 0, 1, kb, inc=s_fd)
        wa = [(s_da, i - 1)] if i >= 2 else ()
        mm(4 + b, slab, 1, 0, kb, waits=wa)
        mm(4 + b + 1, slab, 1, 1, kb, inc=s_fa)

    # --- DVE: per-span is_ge(7.9) indicator with sum-accumulator.
    #     ACT runs ~40ns/span slower, so DVE also takes the second half
    #     of the last ACT span (A15) to equalise the finish times. ---
    DCOL = [slice(0, 1024), slice(1024, 2048)]
    ACOL = [slice(2048, 3072), slice(3072, 4096)]
    for j in range(16):                 # D-span index
        nc.vector.wait_ge(s_fd, j + 1)
        nc.vector.tensor_scalar(run[:, :], ps[:, DCOL[j % 2]], 7.9, 0.0,
                                op0=Alu.is_ge, op1=Alu.add,
                                accum_out=stat[:, j:j + 1]).then_inc(s_dd)
    nc.vector.wait_ge(s_fa, 16)
    nc.vector.tensor_scalar(run[:, 0:512], ps[:, 3584:4096], 7.9, 0.0,
                            op0=Alu.is_ge, op1=Alu.add,
                            accum_out=stat[:, 32:33]).then_inc(s_dd)

    # --- ACT: relu-accumulate over its spans (only half of A15) ---
    nc.scalar.wait_ge(s_ms, 2)          # rbias + stat initialised
    for i in range(16):
        nc.scalar.wait_ge(s_fa, i + 1)
        if i == 15:
            nc.scalar.activation(scr[:, 0:512], ps[:, 3072:3584], AF.Relu,
                                 bias=rbias[:, 0:1], scale=80.0,
                                 accum_out=stat[:, 31:32]).then_inc(s_da)
        else:
            nc.scalar.activation(scr[:, 0:1024], ps[:, ACOL[i % 2]],
                                 AF.Relu, bias=rbias[:, 0:1], scale=80.0,
                                 accum_out=stat[:, 16 + i:17 + i]
                                 ).then_inc(s_da)

    # --- flag (triggered by scalar): after out landed + ALL stats have
    #     been read out (incl. its own accumulator readouts, which walrus
    #     may schedule past later sequencer work); no completion wait ---
    nc.scalar.wait_ge(s_da, N_ASPAN)
    nc.scalar.wait_ge(s_od, 16)
    nc.scalar.wait_ge(s_dd, 17)
    nc.scalar.dma_start(flag[:], stat[:, :]).then_inc(s_od, 16)

    nc.compile()
    return nc


def get_nc():
    if "nc" not in _CACHE:
        _CACHE["nc"] = _build_screen()
    return _CACHE["nc"]


def make_in_maps(query_up, key_up):
    """Sign quantize (exact +-0.5 bf16) + pure layout transforms."""
    import ml_dtypes
    query_up = np.asarray(query_up, dtype=np.float32)
    key_up = np.asarray(key_up, dtype=np.float32)
    bf = ml_dtypes.float8_e4m3
    in_maps = []
    for c in range(N_CORES):
        b, half = c // 2, c % 2
        q = query_up[b, half * HALF:(half + 1) * HALF]       # [1024, 64]
        qT = np.where(q > 0, 0.5, -0.5).astype(bf).T         # [64, 1024]
        qsm = np.ascontiguousarray(np.concatenate([qT, qT], axis=0))
        kT = np.where(key_up[b] > 0, 0.5, -0.5).astype(bf).T  # [64, 2048]
        km = np.concatenate([kT[:, 0:1024], kT[:, 1024:2048]], axis=0)
        # [k 0:512 | q-slab0 | k 512:1024] (see _build_screen)
        ksm = np.ascontiguousarray(np.concatenate(
            [km[:, 0:512], qsm[:, 0:128], km[:, 512:1024]], axis=1))
        in_maps.append({"qs": qsm, "ks": ksm})
    return in_maps


# ---------------------------------------------------------------------------
# Exact kernel (full implementation) -- only compiled and run if the screen
# statistics fire, i.e. some query/key pair shares a 32-bit sign pattern.
# ---------------------------------------------------------------------------


def _build_exact():
    nc = bacc.Bacc("TRN2", target_bir_lowering=False,
                   enable_partition_id=False)
    # qt4[h*64+d, pair*128+p] = q[p*8 + 2*pair + h, d]
    qt4 = nc.dram_tensor("qt4", [128, HALF // 2], f32, kind="ExternalInput")
    # kt4[dup*64+d, j] = k[j, d]
    kt4 = nc.dram_tensor("kt4", [128, L], f32, kind="ExternalInput")
    out = nc.dram_tensor("out", [HALF, K_MAX], i32, kind="ExternalOutput")
    out_pt = out[:].rearrange("(p t) c -> p (t c)", p=128)

    with TileContext(nc) as tc:
        with tc.tile_pool(name="sb", bufs=1) as sb, \
             tc.tile_pool(name="sb2", bufs=3) as sb2, \
             tc.tile_pool(name="ps", bufs=2, space="PSUM") as ps:

            qsb = sb.tile([128, HALF // 2], f32)
            ksb = sb.tile([128, L], f32)
            sqT4 = sb.tile([128, HALF // 2], bf16)
            skT4 = sb.tile([128, L], bf16)
            nc.default_dma_engine.dma_start(ksb[:, 0:1024], kt4[:, 0:1024])
            nc.scalar.dma_start(ksb[:, 1024:2048], kt4[:, 1024:2048])
            nc.default_dma_engine.dma_start(qsb, qt4[:, :])
            nc.vector.tensor_scalar(skT4, ksb, 0.0, 0.5,
                                    op0=Alu.is_gt, op1=Alu.subtract)
            nc.vector.tensor_scalar(sqT4, qsb, 0.0, 0.5,
                                    op0=Alu.is_gt, op1=Alu.subtract)

            out_sb = sb.tile([128, QT * K_MAX], i32)
            nc.gpsimd.memset(out_sb, -1)

            c2i = sb.tile([128, L], i32)   # SENT - j (key j = column)
            nc.gpsimd.iota(c2i, pattern=[[-1, L]], base=int(SENT),
                           channel_multiplier=0)
            c2f = sb.tile([128, L], f32)
            nc.gpsimd.tensor_copy(c2f, c2i)
            negone = sb.tile([128, K_MAX], f32)
            nc.vector.memset(negone, -1.0)
            for t in range(QT):
                base = (t % 2) * 64
                qc = slice((t // 2) * 128, (t // 2) * 128 + 128)
                lhs0 = sqT4[base:base + 32, qc]
                lhs1 = sqT4[base + 32:base + 64, qc]
                val = sb.tile([128, L], f32, tag="val")
                for h in range(2):
                    p0 = ps.tile([128, 1024], f32, tag="g0")
                    p1 = ps.tile([128, 1024], f32, tag="g1")
                    for sblk in range(2):
                        kc = slice(h * 1024 + sblk * 512,
                                   h * 1024 + (sblk + 1) * 512)
                        sl = slice(sblk * 512, (sblk + 1) * 512)
                        nc.tensor.matmul(p0[:, sl], lhsT=lhs0,
                                         rhs=skT4[base:base + 32, kc],
                                         start=True, stop=True,
                                         tile_position=(base, 0))
                        nc.tensor.matmul(p1[:, sl], lhsT=lhs1,
                                         rhs=skT4[base + 32:base + 64, kc],
                                         start=True, stop=True,
                                         tile_position=(base + 32, 0))
                    hsl = slice(h * 1024, (h + 1) * 1024)
                    m0 = sb2.tile([128, 1024], f32, tag="m0")
                    nc.vector.tensor_scalar(m0, p0, THRESH,
                                            None, op0=Alu.is_ge)
                    m1 = sb2.tile([128, 1024], f32, tag="m1")
                    nc.vector.scalar_tensor_tensor(
                        m1, in0=p1, scalar=THRESH, in1=m0,
                        op0=Alu.is_ge, op1=Alu.max)
                    # val = m1 ? -(j) : -SENT  ==  m1*(SENT-j) - SENT
                    nc.vector.tensor_tensor(
                        out=val[:, hsl], in0=m1, in1=c2f[:, hsl],
                        op=Alu.mult)
                    nc.vector.tensor_scalar_add(val[:, hsl], val[:, hsl],
                                                -SENT)
                # 64 smallest j == 64 largest of val, descending
                no = sb.tile([128, K_MAX], f32, tag="no")
                for it8 in range(8):
                    osl = slice(it8 * 8, (it8 + 1) * 8)
                    nc.vector.max(out=no[:, osl], in_=val)
                    nc.vector.match_replace(
                        out=val, in_to_replace=no[:, osl],
                        in_values=val, imm_value=-SENT)
                jv = sb.tile([128, K_MAX], f32, tag="jv")
                nc.vector.tensor_scalar_mul(jv, no, -1.0)  # j or SENT
                msk = sb.tile([128, K_MAX], u32, tag="msk")
                nc.vector.tensor_scalar(msk, jv, 2048.5, None,
                                        op0=Alu.is_ge)
                nc.vector.copy_predicated(jv, msk, negone)
                nc.vector.tensor_copy(
                    out_sb[:, t * K_MAX:(t + 1) * K_MAX], jv)

            nc.default_dma_engine.dma_start(out_pt, out_sb)

    nc.compile()
    return nc


def get_nc_exact():
    if "nc_exact" not in _CACHE:
        _CACHE["nc_exact"] = _build_exact()
    return _CACHE["nc_exact"]


def make_in_maps_exact(query_up, key_up):
    query_up = np.asarray(query_up, dtype=np.float32)
    key_up = np.asarray(key_up, dtype=np.float32)
    in_maps = []
    for c in range(N_CORES):
        b, half = c // 2, c % 2
        q = query_up[b, half * HALF:(half + 1) * HALF]       # [1024, 64]
        qt4 = np.ascontiguousarray(
            q.reshape(128, 4, 2, D).transpose(2, 3, 1, 0).reshape(
                128, HALF // 2))
        kT = key_up[b].T                                     # [64, 2048]
        kt4 = np.ascontiguousarray(np.concatenate([kT, kT], axis=0))
        in_maps.append({"qt4": qt4, "kt4": kt4})
    return in_maps


def _flag_fires(flag):
    return ((flag[:, 0:16] >= 0.5).any() or (flag[:, 32:33] >= 0.5).any()
            or (flag[:, 16:32] >= 1.0).any())


def _run_exact(query_up, key_up):
    """Run the exact kernel in a fresh subprocess: this runtime cannot
    load a second, different NEFF in one process."""
    import os
    import subprocess
    import sys
    import tempfile
    d = tempfile.mkdtemp(prefix="cand_exact_")
    np.save(os.path.join(d, "q.npy"), np.asarray(query_up, dtype=np.float32))
    np.save(os.path.join(d, "k.npy"), np.asarray(key_up, dtype=np.float32))
    here = os.path.dirname(os.path.abspath(__file__))
    code = (
        "import sys, numpy as np\n"
        f"sys.path.insert(0, {here!r})\n"
        "import kernel as K\n"
        "from concourse import bass_utils\n"
        f"q = np.load({os.path.join(d, 'q.npy')!r})\n"
        f"k = np.load({os.path.join(d, 'k.npy')!r})\n"
        "res = bass_utils.run_bass_kernel_spmd(\n"
        "    K.get_nc_exact(), K.make_in_maps_exact(q, k),\n"
        "    core_ids=list(range(K.N_CORES)))\n"
        "np.save(" + repr(os.path.join(d, "out.npy")) + ",\n"
        "        np.stack([res.results[c]['out']\n"
        "                  for c in range(K.N_CORES)]))\n"
    )
    subprocess.run([sys.executable, "-c", code], check=True)
    return np.load(os.path.join(d, "out.npy"))


def kernel(query_up, key_up, head_idx=None, **_ignored):
    nc = get_nc()
    in_maps = make_in_maps(query_up, key_up)
    res = bass_utils.run_bass_kernel_spmd(
        nc, in_maps, core_ids=list(range(N_CORES)))
    full = np.empty((B, L, K_MAX), dtype=np.int32)
    if any(_flag_fires(res.results[c]["flag"]) for c in range(N_CORES)):
        # rare: some pair shares a full 32-bit sign pattern -> exact kernel
        outs = _run_exact(query_up, key_up)
        for c in range(N_CORES):
            b, half = c // 2, c % 2
            full[b, half * HALF:(half + 1) * HALF] = outs[c]
    else:
        for c in range(N_CORES):
            b, half = c // 2, c % 2
            full[b, half * HALF:(half + 1) * HALF] = res.results[c]["out"]
    return full
